# revision 16
# baseline (speedup 1.0000x reference)
"""GF(2) linear block encoder c = (b @ G) mod 2 on 8 TRN2 NeuronCores.

Strategy:
  - Data-parallel: shard b rows (32768 -> 8 x 4096), replicate G.
  - Bits {0,1} are exact in fp8-e4m3 and products accumulate exactly in
    fp32 PSUM, so the GF(2) matmul is an fp8 DoubleRow matmul (K=256 per
    MM) at 2x bf16 throughput -- the PE floor for this shape (~110us).
  - Output is uint16 bits (ACT casts PSUM fp32 -> uint16, DVE ands with
    1), upcast to int32 on the host: 2x less output HBM traffic.
  - dma_start costs ~0.7us of issuing-sequencer time, so pushes are
    budgeted: inputs ride the sync + gpsimd queues only (scalar must
    reach its extraction COPYs immediately or PSUM backpressure stalls
    the PE), b arrives in 4-chunk groups (8KB/partition lines), and G
    is ordered g0-halves, g1..g3 so tile 0 can run kp-outer DMA-paced.
  - PE p-state is pre-warmed with dummy matmuls during the ~9us DMA
    queue startup; the last tile extracts per 512-col PSUM bank so the
    tail is one quarter-extract + one 64KiB DMA.
"""

import sys

import numpy as np

if "/opt/trn_rl_repo" not in sys.path:
    sys.path.insert(0, "/opt/trn_rl_repo")

import ml_dtypes

B_ROWS = 32768
K_MSG = 1024
N_CODE = 2048
NCORES = 8
M = B_ROWS // NCORES  # 4096 rows per core
KS = K_MSG // 128     # 8 k-subtiles of 128
KP = KS // 2          # 4 DoubleRow k-pair steps (K=256 each)
MT = M // 128         # 32 m-tiles
NT = N_CODE // 512    # 4 n-chunks (one PSUM bank each)
MC = 16               # b chunks along m (2 m-tiles each)
MCW = M // MC         # 256 rows per chunk
BG = 4                # b chunks per DMA group
NBG = MC // BG        # 4 groups

F8 = ml_dtypes.float8_e4m3

_NC_CACHE = None


def _build_bass():
    import concourse.bacc as bacc
    import concourse.mybir as mybir
    from concourse import tile

    nc = bacc.Bacc("TRN2", target_bir_lowering=False, debug=False)

    # bt[p, c, s, j] = b bit for row m = c*MCW + j, k = s*128 + p
    bt = nc.dram_tensor("bt", [128, MC, KS, MCW], mybir.dt.float8e4, kind="ExternalInput")
    g = nc.dram_tensor("g", [128, KS, N_CODE], mybir.dt.float8e4, kind="ExternalInput")
    c = nc.dram_tensor("c", [M, N_CODE], mybir.dt.uint16, kind="ExternalOutput")

    dr = mybir.MatmulPerfMode.DoubleRow
    NH = N_CODE // 2

    with tile.TileContext(nc) as tc:
        with (
            tc.tile_pool(name="persist", bufs=1) as persist,
            tc.tile_pool(name="psum", bufs=2, space="PSUM") as psum_pool,
            tc.tile_pool(name="mids", bufs=6) as mids,
        ):
            g_tiles = [
                persist.tile([128, 2, N_CODE], mybir.dt.float8e4, name=f"gt{kp}", tag=f"g{kp}")
                for kp in range(KP)
            ]
            b_groups = [
                persist.tile([128, BG, KS, MCW], mybir.dt.float8e4, name=f"bg{i}", tag=f"bg{i}")
                for i in range(NBG)
            ]

            # --- input pushes. Each DMA ring sustains only ~130-160 GB/s,
            # so inputs are striped across all THREE rings (sync, scalar,
            # SWDGE) in consumption order. scalar's pushes are few and
            # early so they never delay its extraction COPYs.
            def gh(kp, half, eng):
                eng.dma_start(
                    out=g_tiles[kp][:, :, half * NH : (half + 1) * NH],
                    in_=g[:, 2 * kp : 2 * kp + 2, half * NH : (half + 1) * NH],
                )

            # ring 1 (sync):  g0a, g1a, g3a, b4-7, b12-15
            # ring 2 (scalar): g0b, g2a, g3b, b1
            # ring 3 (SWDGE): b0, g1b, g2b, b2-3, b8-11
            nc.gpsimd.dma_start(out=b_groups[0][:, 0:1], in_=bt[:, 0:1, :, :])
            gh(0, 0, nc.sync)
            gh(0, 1, nc.scalar)
            gh(1, 0, nc.sync)
            gh(1, 1, nc.gpsimd)
            gh(2, 0, nc.scalar)
            gh(2, 1, nc.gpsimd)
            gh(3, 0, nc.sync)
            gh(3, 1, nc.scalar)
            nc.scalar.dma_start(out=b_groups[0][:, 1:2], in_=bt[:, 1:2, :, :])
            nc.gpsimd.dma_start(out=b_groups[0][:, 2:BG], in_=bt[:, 2:BG, :, :])
            nc.sync.dma_start(out=b_groups[1], in_=bt[:, BG : 2 * BG, :, :])
            nc.gpsimd.dma_start(out=b_groups[2], in_=bt[:, 2 * BG : 3 * BG, :, :])
            nc.sync.dma_start(out=b_groups[3], in_=bt[:, 3 * BG : 4 * BG, :, :])

            # PE p-state pre-warm with dummy matmuls while queues start up
            zb = persist.tile([128, 2, 128], mybir.dt.float8e4, name="zwarm")
            nc.vector.memset(zb, 0)
            ps_warm = psum_pool.tile([128, N_CODE], mybir.dt.float32, name="ps")
            for w in range(18):
                nc.tensor.matmul(
                    ps_warm[:, 0:128], zb, zb, start=True, stop=True, perf_mode=dr
                )

            # output viewed per m-tile: m = mt*128 + p
            c_view = c.rearrange("(mt p) n -> mt p n", p=128)

            # out-DMA pushes alternate scalar (between its ACTs) and SWDGE
            out_eng = [(nc.gpsimd, nc.scalar)[i % 2] for i in range(MT)]

            def bsta(mt, kp):
                mc, j = mt // 2, mt % 2
                return b_groups[mc // BG][
                    :, mc % BG, 2 * kp : 2 * kp + 2, j * 128 : (j + 1) * 128
                ]

            def mm(ps, mt, kp, nt):
                nc.tensor.matmul(
                    ps[:, nt * 512 : (nt + 1) * 512],
                    bsta(mt, kp),
                    g_tiles[kp][:, :, nt * 512 : (nt + 1) * 512],
                    start=(kp == 0),
                    stop=(kp == KP - 1),
                    perf_mode=dr,
                )

            for mt in range(MT):
                if mt < MT - 1:
                    ps = psum_pool.tile([128, N_CODE], mybir.dt.float32, name="ps")
                    for kp in range(KP):
                        for nt in range(NT):
                            mm(ps, mt, kp, nt)
                    mid = mids.tile([128, N_CODE], mybir.dt.uint16)
                    nc.scalar.activation(mid, ps, mybir.ActivationFunctionType.Copy)
                    nc.vector.tensor_scalar(
                        out=mid,
                        in0=mid,
                        scalar1=1,
                        scalar2=None,
                        op0=mybir.AluOpType.bitwise_and,
                    )
                    out_eng[mt].dma_start(out=c_view[mt], in_=mid)
                else:
                    # last tile: per-bank PSUM tiles, extract + stream out
                    # each 512-col quarter while the PE finishes the rest
                    mid = mids.tile([128, N_CODE], mybir.dt.uint16)
                    for nt in range(NT):
                        psq = psum_pool.tile([128, 512], mybir.dt.float32, name="ps")
                        for kp in range(KP):
                            nc.tensor.matmul(
                                psq,
                                bsta(mt, kp),
                                g_tiles[kp][:, :, nt * 512 : (nt + 1) * 512],
                                start=(kp == 0),
                                stop=(kp == KP - 1),
                                perf_mode=dr,
                            )
                        nc.scalar.activation(
                            mid[:, nt * 512 : (nt + 1) * 512],
                            psq,
                            mybir.ActivationFunctionType.Copy,
                        )
                        nc.vector.tensor_scalar(
                            out=mid[:, nt * 512 : (nt + 1) * 512],
                            in0=mid[:, nt * 512 : (nt + 1) * 512],
                            scalar1=1,
                            scalar2=None,
                            op0=mybir.AluOpType.bitwise_and,
                        )
                        out_eng[mt].dma_start(
                            out=c_view[mt][:, nt * 512 : (nt + 1) * 512],
                            in_=mid[:, nt * 512 : (nt + 1) * 512],
                        )

    nc.finalize()
    return nc


def _get_nc():
    global _NC_CACHE
    if _NC_CACHE is None:
        _NC_CACHE = _build_bass()
    return _NC_CACHE


def _pack_inputs(b, G):
    b8 = np.asarray(b).astype(np.uint8)
    G8 = np.asarray(G).astype(np.uint8)
    # g[p, s, n], k = s*128 + p
    g_f8 = G8.reshape(KS, 128, N_CODE).transpose(1, 0, 2).astype(F8, order="C")
    bts = []
    for core in range(NCORES):
        sh = b8[core * M : (core + 1) * M]  # [M, K]
        # bt[p, c, s, j]: m = c*MCW + j, k = s*128 + p
        btc = sh.reshape(MC, MCW, KS, 128).transpose(3, 0, 2, 1)
        bts.append(btc.astype(F8, order="C"))
    return bts, g_f8


def kernel(b, G, trace=False, **run_kwargs):
    from concourse.bass_utils import run_bass_kernel_spmd

    nc = _get_nc()
    bts, g_f8 = _pack_inputs(b, G)
    in_maps = [{"bt": bts[i], "g": g_f8} for i in range(NCORES)]
    res = run_bass_kernel_spmd(
        nc, in_maps, core_ids=list(range(NCORES)), trace=trace, **run_kwargs
    )
    out = np.concatenate([res.results[i]["c"] for i in range(NCORES)], axis=0)
    out = out.astype(np.int32)
    if trace:
        kernel.last_results = res
    return out


kernel.last_results = None


# revision 17
# speedup vs baseline: 1.0074x; 1.0074x over previous
"""GF(2) linear block encoder c = (b @ G) mod 2 on 8 TRN2 NeuronCores.

Strategy:
  - Data-parallel: shard b rows (32768 -> 8 x 4096), replicate G.
  - Bits {0,1} are exact in fp8-e4m3 and products accumulate exactly in
    fp32 PSUM, so the GF(2) matmul is an fp8 DoubleRow matmul (K=256 per
    MM) at 2x bf16 throughput -- the PE floor for this shape (~110us).
  - Output is uint16 bits (ACT casts PSUM fp32 -> uint16, DVE ands with
    1), upcast to int32 on the host: 2x less output HBM traffic.
  - dma_start costs ~0.7us of issuing-sequencer time, so pushes are
    budgeted: inputs ride the sync + gpsimd queues only (scalar must
    reach its extraction COPYs immediately or PSUM backpressure stalls
    the PE), b arrives in 4-chunk groups (8KB/partition lines), and G
    is ordered g0-halves, g1..g3 so tile 0 can run kp-outer DMA-paced.
  - PE p-state is pre-warmed with dummy matmuls during the ~9us DMA
    queue startup; the last tile extracts per 512-col PSUM bank so the
    tail is one quarter-extract + one 64KiB DMA.
"""

import sys

import numpy as np

if "/opt/trn_rl_repo" not in sys.path:
    sys.path.insert(0, "/opt/trn_rl_repo")

import ml_dtypes

B_ROWS = 32768
K_MSG = 1024
N_CODE = 2048
NCORES = 8
M = B_ROWS // NCORES  # 4096 rows per core
KS = K_MSG // 128     # 8 k-subtiles of 128
KP = KS // 2          # 4 DoubleRow k-pair steps (K=256 each)
MT = M // 128         # 32 m-tiles
NT = N_CODE // 512    # 4 n-chunks (one PSUM bank each)
MC = 16               # b chunks along m (2 m-tiles each)
MCW = M // MC         # 256 rows per chunk
BG = 4                # b chunks per DMA group
NBG = MC // BG        # 4 groups

F8 = ml_dtypes.float8_e4m3

_NC_CACHE = None


def _build_bass():
    import concourse.bacc as bacc
    import concourse.mybir as mybir
    from concourse import tile

    nc = bacc.Bacc("TRN2", target_bir_lowering=False, debug=False)

    # bt[p, c, s, j] = b bit for row m = c*MCW + j, k = s*128 + p
    bt = nc.dram_tensor("bt", [128, MC, KS, MCW], mybir.dt.float8e4, kind="ExternalInput")
    g = nc.dram_tensor("g", [128, KS, N_CODE], mybir.dt.float8e4, kind="ExternalInput")
    c = nc.dram_tensor("c", [M, N_CODE], mybir.dt.uint8, kind="ExternalOutput")

    dr = mybir.MatmulPerfMode.DoubleRow
    NH = N_CODE // 2

    with tile.TileContext(nc) as tc:
        with (
            tc.tile_pool(name="persist", bufs=1) as persist,
            tc.tile_pool(name="psum", bufs=2, space="PSUM") as psum_pool,
            tc.tile_pool(name="mids", bufs=6) as mids,
            tc.tile_pool(name="c8s", bufs=6) as c8s,
        ):
            g_tiles = [
                persist.tile([128, 2, N_CODE], mybir.dt.float8e4, name=f"gt{kp}", tag=f"g{kp}")
                for kp in range(KP)
            ]
            b_groups = [
                persist.tile([128, BG, KS, MCW], mybir.dt.float8e4, name=f"bg{i}", tag=f"bg{i}")
                for i in range(NBG)
            ]

            # --- input pushes. Each DMA ring sustains only ~130-160 GB/s,
            # so inputs are striped across all THREE rings (sync, scalar,
            # SWDGE) in consumption order. scalar's pushes are few and
            # early so they never delay its extraction COPYs.
            def gh(kp, half, eng):
                eng.dma_start(
                    out=g_tiles[kp][:, :, half * NH : (half + 1) * NH],
                    in_=g[:, 2 * kp : 2 * kp + 2, half * NH : (half + 1) * NH],
                )

            # ring 1 (sync):  g0a, g1a, g3a, b4-7, b12-15
            # ring 2 (scalar): g0b, g2a, g3b, b1
            # ring 3 (SWDGE): b0, g1b, g2b, b2-3, b8-11
            nc.gpsimd.dma_start(out=b_groups[0][:, 0:1], in_=bt[:, 0:1, :, :])
            gh(0, 0, nc.sync)
            gh(0, 1, nc.scalar)
            gh(1, 0, nc.sync)
            gh(1, 1, nc.gpsimd)
            gh(2, 0, nc.scalar)
            gh(2, 1, nc.gpsimd)
            gh(3, 0, nc.sync)
            gh(3, 1, nc.scalar)
            nc.scalar.dma_start(out=b_groups[0][:, 1:2], in_=bt[:, 1:2, :, :])
            nc.gpsimd.dma_start(out=b_groups[0][:, 2:BG], in_=bt[:, 2:BG, :, :])
            nc.sync.dma_start(out=b_groups[1], in_=bt[:, BG : 2 * BG, :, :])
            nc.gpsimd.dma_start(out=b_groups[2], in_=bt[:, 2 * BG : 3 * BG, :, :])
            nc.sync.dma_start(out=b_groups[3], in_=bt[:, 3 * BG : 4 * BG, :, :])

            # PE p-state pre-warm with dummy matmuls while queues start up
            zb = persist.tile([128, 2, 128], mybir.dt.float8e4, name="zwarm")
            nc.vector.memset(zb, 0)
            ps_warm = psum_pool.tile([128, N_CODE], mybir.dt.float32, name="ps")
            for w in range(26):
                nc.tensor.matmul(
                    ps_warm[:, 0:128], zb, zb, start=True, stop=True, perf_mode=dr
                )

            # output viewed per m-tile: m = mt*128 + p
            c_view = c.rearrange("(mt p) n -> mt p n", p=128)

            # out-DMA pushes alternate scalar (between its ACTs) and SWDGE
            out_eng = [(nc.gpsimd, nc.scalar)[i % 2] for i in range(MT)]

            def bsta(mt, kp):
                mc, j = mt // 2, mt % 2
                return b_groups[mc // BG][
                    :, mc % BG, 2 * kp : 2 * kp + 2, j * 128 : (j + 1) * 128
                ]

            def mm(ps, mt, kp, nt):
                nc.tensor.matmul(
                    ps[:, nt * 512 : (nt + 1) * 512],
                    bsta(mt, kp),
                    g_tiles[kp][:, :, nt * 512 : (nt + 1) * 512],
                    start=(kp == 0),
                    stop=(kp == KP - 1),
                    perf_mode=dr,
                )

            for mt in range(MT):
                if mt < MT - 1:
                    ps = psum_pool.tile([128, N_CODE], mybir.dt.float32, name="ps")
                    for kp in range(KP):
                        for nt in range(NT):
                            mm(ps, mt, kp, nt)
                    mid = mids.tile([128, N_CODE], mybir.dt.uint16)
                    c8 = c8s.tile([128, N_CODE], mybir.dt.uint8)
                    nc.scalar.activation(mid, ps, mybir.ActivationFunctionType.Copy)
                    nc.vector.tensor_scalar(
                        out=mid,
                        in0=mid,
                        scalar1=1,
                        scalar2=None,
                        op0=mybir.AluOpType.bitwise_and,
                    )
                    # narrowing cast uint16 {0,1} -> uint8, alternating the
                    # engine so neither ACT nor DVE exceeds the PE pace
                    if mt % 2 == 0:
                        nc.vector.tensor_scalar(
                            out=c8,
                            in0=mid,
                            scalar1=0,
                            scalar2=None,
                            op0=mybir.AluOpType.bypass,
                        )
                    else:
                        nc.scalar.copy(c8, mid)
                    out_eng[mt].dma_start(out=c_view[mt], in_=c8)
                else:
                    # last tile: per-bank PSUM tiles, extract + stream out
                    # each 512-col quarter while the PE finishes the rest
                    mid = mids.tile([128, N_CODE], mybir.dt.uint16)
                    c8 = c8s.tile([128, N_CODE], mybir.dt.uint8)
                    for nt in range(NT):
                        psq = psum_pool.tile([128, 512], mybir.dt.float32, name="ps")
                        for kp in range(KP):
                            nc.tensor.matmul(
                                psq,
                                bsta(mt, kp),
                                g_tiles[kp][:, :, nt * 512 : (nt + 1) * 512],
                                start=(kp == 0),
                                stop=(kp == KP - 1),
                                perf_mode=dr,
                            )
                        nc.scalar.activation(
                            mid[:, nt * 512 : (nt + 1) * 512],
                            psq,
                            mybir.ActivationFunctionType.Copy,
                        )
                        nc.vector.tensor_scalar(
                            out=mid[:, nt * 512 : (nt + 1) * 512],
                            in0=mid[:, nt * 512 : (nt + 1) * 512],
                            scalar1=1,
                            scalar2=None,
                            op0=mybir.AluOpType.bitwise_and,
                        )
                        nc.vector.tensor_scalar(
                            out=c8[:, nt * 512 : (nt + 1) * 512],
                            in0=mid[:, nt * 512 : (nt + 1) * 512],
                            scalar1=0,
                            scalar2=None,
                            op0=mybir.AluOpType.bypass,
                        )
                        out_eng[mt].dma_start(
                            out=c_view[mt][:, nt * 512 : (nt + 1) * 512],
                            in_=c8[:, nt * 512 : (nt + 1) * 512],
                        )

    nc.finalize()
    return nc


def _get_nc():
    global _NC_CACHE
    if _NC_CACHE is None:
        _NC_CACHE = _build_bass()
    return _NC_CACHE


def _pack_inputs(b, G):
    b8 = np.asarray(b).astype(np.uint8)
    G8 = np.asarray(G).astype(np.uint8)
    # g[p, s, n], k = s*128 + p
    g_f8 = G8.reshape(KS, 128, N_CODE).transpose(1, 0, 2).astype(F8, order="C")
    bts = []
    for core in range(NCORES):
        sh = b8[core * M : (core + 1) * M]  # [M, K]
        # bt[p, c, s, j]: m = c*MCW + j, k = s*128 + p
        btc = sh.reshape(MC, MCW, KS, 128).transpose(3, 0, 2, 1)
        bts.append(btc.astype(F8, order="C"))
    return bts, g_f8


def kernel(b, G, trace=False, **run_kwargs):
    from concourse.bass_utils import run_bass_kernel_spmd

    nc = _get_nc()
    bts, g_f8 = _pack_inputs(b, G)
    in_maps = [{"bt": bts[i], "g": g_f8} for i in range(NCORES)]
    res = run_bass_kernel_spmd(
        nc, in_maps, core_ids=list(range(NCORES)), trace=trace, **run_kwargs
    )
    out = np.concatenate([res.results[i]["c"] for i in range(NCORES)], axis=0)
    out = out.astype(np.int32)
    if trace:
        kernel.last_results = res
    return out


kernel.last_results = None
